# revision 17
# baseline (speedup 1.0000x reference)
"""BiRGAT (bipartite 2-layer GATv2) Trainium2 kernel, 8-core SPMD.

Destination-tile sharding (as v1) with a restructured bf16 edge phase:
- Tables (layer-1/3 transforms) stored bf16 with 4 appended columns holding
  0.2*(att . row) partial dots, so the leaky-relu attention score splits as
  alpha = 0.8*(att . relu(v)) + 0.2*(att . v) with the linear part gathered
  for free (lrelu(v) = 0.2 v + 0.8 relu(v)).
- One-hot scatter matrices (onehot [e,d] and its transpose [d,e]) are
  precomputed on host per chunk and DMA'd (bf16), removing the per-chunk
  iota/is_equal/PE-transpose/PSUM-copy chain.
- Per chunk: indirect-DMA gather of 260/516-wide source rows; v^T built on
  PE via transpose-matmul (rhs=identity) + xr scatter-matmul (rhs=onehotT)
  into PSUM; relu (Act/DVE alternating) -> rT bf16; alpha on PE
  (lhsT=rT k-tiles, rhs=0.8-scaled block-diagonal att + linear-part mms);
  exp batched over 4 chunks into msgs[:, w:w+4]; one 4D DVE multiply forms
  msgs; one-hot scatter matmul accumulates [messages | ea] per slot in PSUM.
- Softmax normalization per slot in fp32 (no segment_max; alpha std ~0.3).
"""
import sys

sys.path.insert(0, "/opt/trn_rl_repo")

import numpy as np
import ml_dtypes
from contextlib import ExitStack

import concourse.bass as bass
import concourse.tile as tile
from concourse import bacc, mybir
from concourse.bass_utils import run_bass_kernel_spmd
from concourse.masks import make_identity

P = 128
NCORES = 8
NS, NG, E = 4096, 20000, 131072
DIN, H, C1, C3 = 256, 4, 64, 128
HC1, HC3 = H * C1, H * C3          # 256, 512
W1, W3 = HC1 + H, HC3 + H         # 260, 516 (table row widths)
NGP = 20480                        # genes padded to 160 tiles
NST, NGT = NS // P, NGP // P       # 32, 160
S_PER_CORE, G_PER_CORE = NST // NCORES, NGT // NCORES   # 4, 20
SROWS, GROWS = S_PER_CORE * P, G_PER_CORE * P           # 512, 2560

F32 = mybir.dt.float32
BF16 = mybir.dt.bfloat16
I32 = mybir.dt.int32
AF = mybir.ActivationFunctionType
OP = mybir.AluOpType
BF = ml_dtypes.bfloat16


# ---------------------------------------------------------------- host plan

def _deal_tiles(dst, n_tiles, per_core):
    tcnt = np.bincount(dst // P, minlength=n_tiles)
    chunks = np.maximum((tcnt + P - 1) // P, 1)
    order = np.argsort(-chunks, kind="stable")
    assign = order.reshape(per_core, NCORES)
    sched = chunks[assign].max(axis=1)
    return assign, sched.astype(int)


def _edge_arrays(src, dst, assign, sched, src_row_map, core):
    """srcr [128, nch] i32 table-row per edge; ohp [128, nch, 2, 128] bf16
    with [:, c, 0, :] = onehot[e, d] and [:, c, 1, :] = onehotT[d, e]."""
    nch = int(sched.sum())
    srcr = np.zeros((P, nch), np.int32)
    ohp = np.zeros((P, nch, 2, P), BF)
    tile_of = dst // P
    ci = 0
    for slot in range(len(sched)):
        t = assign[slot, core]
        e = np.nonzero(tile_of == t)[0]
        n = len(e)
        rows = src_row_map[src[e]]
        dl = (dst[e] % P).astype(np.int64)
        for k in range(int(sched[slot])):
            lo, hi = k * P, min((k + 1) * P, n)
            cnt = hi - lo
            if cnt > 0:
                srcr[:cnt, ci] = rows[lo:hi]
                ohp[np.arange(cnt), ci, 0, dl[lo:hi]] = 1.0
                ohp[dl[lo:hi], ci, 1, np.arange(cnt)] = 1.0
            ci += 1
    return srcr, ohp


def _rhs(w):
    """[K, N] -> [128, K//128, N] rhs tile layout."""
    K, N = w.shape
    return np.ascontiguousarray(
        w.reshape(K // P, P, N).transpose(1, 0, 2)).astype(BF)


def _xt(x):
    """[R, K] -> [128, K//128, R] lhsT layout (features on partitions)."""
    R, K = x.shape
    return np.ascontiguousarray(
        x.T.reshape(K // P, P, R).transpose(1, 0, 2)).astype(BF)


def _ext_w(Wl, bl, att, out_c):
    """Extend [K, H*out_c] weight with 4 cols of 0.2*(att . W) partial dots.
    Returns (W_ext [K, H*out_c+4] f32, b_ext [H*out_c+4] f32)."""
    K = Wl.shape[0]
    Wp = np.einsum('khc,hc->kh', Wl.reshape(K, H, out_c), att) * 0.2
    bp = np.einsum('hc,hc->h', bl.reshape(H, out_c), att) * 0.2
    return (np.concatenate([Wl, Wp], axis=1),
            np.concatenate([bl, bp], axis=0))


def _attblk(att, out_c, kc):
    """[128, kc, H] bf16 block-diagonal attention, scaled by 0.8."""
    a = np.zeros((P, kc, H), np.float32)
    for k in range(kc):
        for p in range(P):
            f = k * P + p
            a[p, k, f // out_c] = 0.8 * att[f // out_c, f % out_c]
    return a.astype(BF)


def _bcast(v):
    v = np.asarray(v, np.float32).reshape(-1)
    return np.broadcast_to(v[None, :], (P, len(v))).copy()


def _plan(inputs):
    f32 = lambda k: np.asarray(inputs[k], np.float32)
    sg_src = np.asarray(inputs["sg_src"]); sg_dst = np.asarray(inputs["sg_dst"])
    gs_src = np.asarray(inputs["gs_src"]); gs_dst = np.asarray(inputs["gs_dst"])

    g_assign, g_sched = _deal_tiles(sg_dst, NGT, G_PER_CORE)
    s_assign, s_sched = _deal_tiles(gs_dst, NST, S_PER_CORE)

    g_owner = np.zeros(NGT, np.int64); g_slot = np.zeros(NGT, np.int64)
    for slot in range(G_PER_CORE):
        for c in range(NCORES):
            g_owner[g_assign[slot, c]] = c
            g_slot[g_assign[slot, c]] = slot
    s_owner = np.zeros(NST, np.int64); s_slot = np.zeros(NST, np.int64)
    for slot in range(S_PER_CORE):
        for c in range(NCORES):
            s_owner[s_assign[slot, c]] = c
            s_slot[s_assign[slot, c]] = slot

    sid = np.arange(NS)
    srow_tbl = s_owner[sid // P] * SROWS + s_slot[sid // P] * P + sid % P
    gid = np.arange(NG)
    grow_tbl = g_owner[gid // P] * GROWS + g_slot[gid // P] * P + gid % P

    att1_sg = f32("att1_sg"); att1_gs = f32("att1_gs"); att3 = f32("att3_gs")

    Wl1_sg_e, bl1_sg_e = _ext_w(f32("Wl1_sg"), f32("bl1_sg"), att1_sg, C1)
    Wr1_sg_e, br1_sg_e = _ext_w(f32("Wr1_sg"), f32("br1_sg"), att1_sg, C1)
    Wl1_gs_e, bl1_gs_e = _ext_w(f32("Wl1_gs"), f32("bl1_gs"), att1_gs, C1)
    Wr1_gs_e, br1_gs_e = _ext_w(f32("Wr1_gs"), f32("br1_gs"), att1_gs, C1)
    Wl3_e, bl3_e = _ext_w(f32("Wl3_gs"), f32("bl3_gs"), att3, C3)
    Wr3_e, br3_e = _ext_w(f32("Wr3_gs"), f32("br3_gs"), att3, C3)

    x_sample = f32("x_sample"); x_gene = f32("x_gene")

    in_maps = []
    for c in range(NCORES):
        xs_own = x_sample.reshape(NST, P, DIN)[s_assign[:, c]].reshape(SROWS, DIN)
        xg_own = np.zeros((GROWS, DIN), np.float32)
        for i, t in enumerate(g_assign[:, c]):
            lo = t * P
            if lo < NG:
                n = min(P, NG - lo)
                xg_own[i * P:i * P + n] = x_gene[lo:lo + n]

        sg_srcr, sg_ohp = _edge_arrays(sg_src, sg_dst, g_assign, g_sched,
                                       srow_tbl, c)
        gs_srcr, gs_ohp = _edge_arrays(gs_src, gs_dst, s_assign, s_sched,
                                       grow_tbl, c)

        m = {
            "xsT": _xt(xs_own), "xgT": _xt(xg_own),
            "Wl1_sg": _rhs(Wl1_sg_e), "Wr1_sg": _rhs(Wr1_sg_e),
            "Wl1_gs": _rhs(Wl1_gs_e), "Wr1_gs": _rhs(Wr1_gs_e),
            "Wl3": _rhs(Wl3_e), "Wr3": _rhs(Wr3_e),
            "sl1_W": _rhs(f32("sl1_W")), "sl3_W": _rhs(f32("sl3_W")),
            "bl1_sg_b": _bcast(bl1_sg_e).astype(BF),
            "br1_sg_b": _bcast(br1_sg_e).astype(BF),
            "bl1_gs_b": _bcast(bl1_gs_e).astype(BF),
            "br1_gs_b": _bcast(br1_gs_e).astype(BF),
            "bl3_b": _bcast(bl3_e).astype(BF),
            "br3_b": _bcast(br3_e).astype(BF),
            "ab1_sg": _attblk(att1_sg, C1, 2),
            "ab1_gs": _attblk(att1_gs, C1, 2),
            "ab3": _attblk(att3, C3, 4),
            "bias1_sg_b": _bcast(inputs["bias1_sg"]),
            "bias1_gs_b": _bcast(inputs["bias1_gs"]),
            "bias3_b": _bcast(inputs["bias3_gs"]),
            "sl1_b_b": _bcast(inputs["sl1_b"]),
            "sl3_b_b": _bcast(inputs["sl3_b"]),
            "identb": np.eye(P, dtype=BF),
            "sg_srcr": sg_srcr, "gs_srcr": gs_srcr,
            "sg_ohp": sg_ohp, "gs_ohp": gs_ohp,
        }
        in_maps.append(m)
    zb = not any(np.any(np.asarray(inputs[k])) for k in
                 ("bl1_sg", "br1_sg", "bl1_gs", "br1_gs", "bl3_gs", "br3_gs",
                  "bias1_sg", "bias1_gs", "bias3_gs", "sl1_b", "sl3_b"))
    plan = {"g_sched": g_sched, "s_sched": s_sched, "s_assign": s_assign,
            "zb": zb}
    return plan, in_maps


# ------------------------------------------------------------- device build

def _ap3(base_ap, h, c, mid, inner):
    return bass.AP(base_ap.tensor, base_ap.offset,
                   [[base_ap.ap[0][0], P], [mid, h], [inner, c]])


def _ap4(base_ap, n, h, c, d0, mid, inner):
    return bass.AP(base_ap.tensor, base_ap.offset,
                   [[base_ap.ap[0][0], P], [d0, n], [mid, h], [inner, c]])


def _build(g_sched, s_sched, zb):
    nsg = int(g_sched.sum())
    ngs = int(s_sched.sum())
    nc = bacc.Bacc("TRN2", target_bir_lowering=False, debug=False,
                   num_devices=NCORES)

    ei = lambda name, shape, dt=BF16: nc.dram_tensor(name, shape, dt,
                                                     kind="ExternalInput")
    xsT = ei("xsT", [P, 2, SROWS]); xgT = ei("xgT", [P, 2, GROWS])
    Wl1_sg = ei("Wl1_sg", [P, 2, W1]); Wr1_sg = ei("Wr1_sg", [P, 2, W1])
    Wl1_gs = ei("Wl1_gs", [P, 2, W1]); Wr1_gs = ei("Wr1_gs", [P, 2, W1])
    Wl3 = ei("Wl3", [P, 2, W3]); Wr3 = ei("Wr3", [P, 2, W3])
    sl1_W = ei("sl1_W", [P, 2, C1]); sl3_W = ei("sl3_W", [P, 2, C3])
    bl1_sg_b = ei("bl1_sg_b", [P, W1]); br1_sg_b = ei("br1_sg_b", [P, W1])
    bl1_gs_b = ei("bl1_gs_b", [P, W1]); br1_gs_b = ei("br1_gs_b", [P, W1])
    bl3_b = ei("bl3_b", [P, W3]); br3_b = ei("br3_b", [P, W3])
    ab1_sg = ei("ab1_sg", [P, 2, H]); ab1_gs = ei("ab1_gs", [P, 2, H])
    ab3 = ei("ab3", [P, 4, H])
    bias1_sg_b = ei("bias1_sg_b", [P, HC1], F32)
    bias1_gs_b = ei("bias1_gs_b", [P, HC1], F32)
    bias3_b = ei("bias3_b", [P, C3], F32)
    sl1_b_b = ei("sl1_b_b", [P, C1], F32)
    sl3_b_b = ei("sl3_b_b", [P, C3], F32)
    identb = ei("identb", [P, P])
    sg_srcr = ei("sg_srcr", [P, nsg], I32)
    gs_srcr = ei("gs_srcr", [P, ngs], I32)
    sg_ohp = ei("sg_ohp", [P, nsg, 2, P])
    gs_ohp = ei("gs_ohp", [P, ngs, 2, P])

    out_own = nc.dram_tensor("out_own", [SROWS, C3], F32, kind="ExternalOutput")

    agin_s = nc.dram_tensor("agin_s", [SROWS, W1], BF16)
    agin_g = nc.dram_tensor("agin_g", [GROWS, W1], BF16)
    agin_3 = nc.dram_tensor("agin_3", [GROWS, W3], BF16)
    tbl_s = nc.dram_tensor("tbl_s", [NS, W1], BF16, addr_space="Shared")
    tbl_g = nc.dram_tensor("tbl_g", [NGP, W1], BF16, addr_space="Shared")
    tbl_3 = nc.dram_tensor("tbl_3", [NGP, W3], BF16, addr_space="Shared")
    xr1_sg = nc.dram_tensor("xr1_sg", [GROWS, W1], BF16)
    xr1_gs = nc.dram_tensor("xr1_gs", [SROWS, W1], BF16)
    xr3 = nc.dram_tensor("xr3", [SROWS, W3], BF16)

    RG = [list(range(NCORES))]

    with tile.TileContext(nc) as tc, ExitStack() as ctx:
        res = ctx.enter_context(tc.tile_pool(name="res", bufs=1))
        sb = ctx.enter_context(tc.tile_pool(name="sb", bufs=4))
        ev = ctx.enter_context(tc.tile_pool(name="ev", bufs=3))
        xlp = ctx.enter_context(tc.tile_pool(name="xlp", bufs=8))
        ohpool = ctx.enter_context(tc.tile_pool(name="ohpool", bufs=8))
        msp = ctx.enter_context(tc.tile_pool(name="msp", bufs=6))
        rtp = ctx.enter_context(tc.tile_pool(name="rtp", bufs=8))
        xrs = ctx.enter_context(tc.tile_pool(name="xrs", bufs=3))
        psv = ctx.enter_context(tc.tile_pool(name="psv", bufs=4, space="PSUM"))
        psa = ctx.enter_context(tc.tile_pool(name="psa", bufs=2, space="PSUM"))
        pse = ctx.enter_context(tc.tile_pool(name="pse", bufs=1, space="PSUM"))
        psd = ctx.enter_context(tc.tile_pool(name="psd", bufs=1, space="PSUM"))

        def rload(name, dram, shape, dt=BF16):
            t = res.tile(shape, dt, tag=name)
            nc.sync.dma_start(t[:], dram[:])
            return t

        identb_t = rload("identb", identb, [P, P])
        ident32 = res.tile([P, P], F32, tag="ident32")
        make_identity(nc, ident32[:])
        xsT_t = rload("xsT", xsT, [P, 2, SROWS])
        xgT_t = rload("xgT", xgT, [P, 2, GROWS])
        Wl1_sg_t = rload("Wl1_sg", Wl1_sg, [P, 2, W1])
        Wr1_sg_t = rload("Wr1_sg", Wr1_sg, [P, 2, W1])
        Wl1_gs_t = rload("Wl1_gs", Wl1_gs, [P, 2, W1])
        Wr1_gs_t = rload("Wr1_gs", Wr1_gs, [P, 2, W1])
        Wl3_t = rload("Wl3", Wl3, [P, 2, W3])
        Wr3_t = rload("Wr3", Wr3, [P, 2, W3])
        sl1_W_t = rload("sl1_W", sl1_W, [P, 2, C1])
        sl3_W_t = rload("sl3_W", sl3_W, [P, 2, C3])
        bl1_sg_t = rload("bl1_sg", bl1_sg_b, [P, W1])
        br1_sg_t = rload("br1_sg", br1_sg_b, [P, W1])
        bl1_gs_t = rload("bl1_gs", bl1_gs_b, [P, W1])
        br1_gs_t = rload("br1_gs", br1_gs_b, [P, W1])
        bl3_t = rload("bl3", bl3_b, [P, W3])
        br3_t = rload("br3", br3_b, [P, W3])
        ab1_sg_t = rload("ab1_sg", ab1_sg, [P, 2, H])
        ab1_gs_t = rload("ab1_gs", ab1_gs, [P, 2, H])
        ab3_t = rload("ab3", ab3, [P, 4, H])
        bias1_sg_t = rload("bias1_sg", bias1_sg_b, [P, HC1], F32)
        bias1_gs_t = rload("bias1_gs", bias1_gs_b, [P, HC1], F32)
        bias3_t = rload("bias3", bias3_b, [P, C3], F32)
        sl1_b_t = rload("sl1_b", sl1_b_b, [P, C1], F32)
        sl3_b_t = rload("sl3_b", sl3_b_b, [P, C3], F32)
        sg_srcr_t = rload("sg_srcr", sg_srcr, [P, nsg], I32)
        gs_srcr_t = rload("gs_srcr", gs_srcr, [P, ngs], I32)

        sl1_sb = res.tile([P, S_PER_CORE * C1], F32, tag="sl1_sb")
        sl3_sb = res.tile([P, S_PER_CORE * C3], F32, tag="sl3_sb")

        # ---- phase A: node tables (xT shipped pre-transposed from host)
        def dense(xT_t, lo, W_t, n, bias_t, dst, tag):
            """n <= 512: single-bank psum; n > 512: split main/extra cols."""
            nmain = min(n, HC3)
            pt = pse.tile([P, nmain], F32, space="PSUM", tag="pm")
            for k in range(2):
                nc.tensor.matmul(pt[:], lhsT=xT_t[:, k, lo:lo + P],
                                 rhs=W_t[:, k, :nmain], start=(k == 0),
                                 stop=(k == 1))
            o = sb.tile([P, n], BF16, tag=tag)
            nc.vector.tensor_tensor(out=o[:, :nmain], in0=pt[:],
                                    in1=bias_t[:, :nmain], op=OP.add)
            if n > nmain:
                px = psa.tile([P, n - nmain], F32, space="PSUM", tag="pa")
                for k in range(2):
                    nc.tensor.matmul(px[:], lhsT=xT_t[:, k, lo:lo + P],
                                     rhs=W_t[:, k, nmain:n], start=(k == 0),
                                     stop=(k == 1))
                nc.vector.tensor_tensor(out=o[:, nmain:n], in0=px[:],
                                        in1=bias_t[:, nmain:n], op=OP.add)
            nc.sync.dma_start(dst, o[:])

        for i in range(S_PER_CORE):
            lo = i * P
            dense(xsT_t, lo, Wl1_sg_t, W1, bl1_sg_t,
                  agin_s[lo:lo + P, :], "da")
            dense(xsT_t, lo, Wr1_gs_t, W1, br1_gs_t,
                  xr1_gs[lo:lo + P, :], "dx")
            pt = psa.tile([P, C1], F32, space="PSUM", tag="pa")
            for k in range(2):
                nc.tensor.matmul(pt[:], lhsT=xsT_t[:, k, lo:lo + P],
                                 rhs=sl1_W_t[:, k, :], start=(k == 0),
                                 stop=(k == 1))
            if zb:
                nc.scalar.copy(sl1_sb[:, i * C1:(i + 1) * C1], pt[:])
            else:
                nc.vector.tensor_tensor(out=sl1_sb[:, i * C1:(i + 1) * C1],
                                        in0=pt[:], in1=sl1_b_t[:], op=OP.add)
        nc.gpsimd.collective_compute("AllGather", OP.bypass, replica_groups=RG,
                                     ins=[agin_s[:]], outs=[tbl_s[:]])

        for j in range(G_PER_CORE):
            lo = j * P
            dense(xgT_t, lo, Wl1_gs_t, W1, bl1_gs_t,
                  agin_g[lo:lo + P, :], "da")
            dense(xgT_t, lo, Wr1_sg_t, W1, br1_sg_t,
                  xr1_sg[lo:lo + P, :], "dx")
        nc.gpsimd.collective_compute("AllGather", OP.bypass, replica_groups=RG,
                                     ins=[agin_g[:]], outs=[tbl_g[:]])

        # ---- edge phase helper -------------------------------------------
        relu_ctr = [0]

        def edge_slot(ci0, nch, srcr_t, ohp_dram, tbl, xr_slot, ab_t, kc,
                      pm, pd, w, wt):
            """Process chunks ci0..ci0+nch-1 of one slot.
            kc: feature k-tiles (2 or 4); w: message width (256/512);
            wt: table row width (260/516). pm [P, w(+H)] psum accumulates
            messages (+ea for w=256); pd [P, H] psum for w=512."""
            first = True
            nq = (nch + 3) // 4
            for q in range(nq):
                qn = min(4, nch - q * 4)
                cq = ci0 + q * 4
                # batched onehot-pair load (SP queue)
                oh = ohpool.tile([P, 4, 2, P], BF16, tag="oh")
                nc.sync.dma_start(oh[:, :qn, :, :],
                                  ohp_dram[:, cq:cq + qn, :, :])
                xl = xlp.tile([P, 4, wt], BF16, tag=f"xl{wt}")
                for i in range(qn):
                    nc.gpsimd.indirect_dma_start(
                        out=xl[:, i, :], out_offset=None, in_=tbl[:],
                        in_offset=bass.IndirectOffsetOnAxis(
                            ap=srcr_t[:, cq + i:cq + i + 1], axis=0))
                pa = psa.tile([P, 4, H], F32, space="PSUM", tag="pa")
                msgs = msp.tile([P, 4, wt], BF16, tag=f"ms{wt}")
                for i0 in range(0, qn, 2):
                    pn = min(2, qn - i0)
                    if w == HC1:
                        vt = psv.tile([P, 2, kc, P], F32, space="PSUM",
                                      tag="vt")
                        vts = [vt[:, i, :, :] for i in range(pn)]
                    else:
                        vts = []
                        for i in range(pn):
                            v = psv.tile([P, 1, kc, P], F32, space="PSUM",
                                         tag="vt")
                            vts.append(v[:, 0, :, :])
                    for i in range(pn):
                        c = i0 + i
                        # vT = xl^T + onehotT @ xr (kc tiles, one psum bank)
                        for k in range(kc):
                            nc.tensor.matmul(vts[i][:, k, :],
                                             lhsT=xl[:, c, k * P:(k + 1) * P],
                                             rhs=identb_t[:], start=True,
                                             stop=False)
                            nc.tensor.matmul(vts[i][:, k, :],
                                             lhsT=xr_slot[:, k * P:(k + 1) * P],
                                             rhs=oh[:, c, 1, :], start=False,
                                             stop=True)
                        # alpha linear part (0.2-scaled table/xr columns)
                        nc.tensor.matmul(pa[:, c, :], lhsT=identb_t[:],
                                         rhs=xl[:, c, w:w + H], start=True,
                                         stop=False)
                        nc.tensor.matmul(pa[:, c, :], lhsT=oh[:, c, 1, :],
                                         rhs=xr_slot[:, w:w + H], start=False,
                                         stop=False)
                    rt = rtp.tile([P, 2, kc, P], BF16, tag="rt")
                    rc = relu_ctr[0]; relu_ctr[0] += 1
                    if w == HC1 and pn == 2:
                        if rc % 5 < 2:
                            nc.vector.tensor_scalar(out=rt[:], in0=vt[:],
                                                    scalar1=0.0, scalar2=None,
                                                    op0=OP.max)
                        else:
                            nc.scalar.activation(rt[:], vt[:], AF.Relu)
                    else:
                        for i in range(pn):
                            if rc % 5 < 2:
                                nc.vector.tensor_scalar(out=rt[:, i, :, :],
                                                        in0=vts[i],
                                                        scalar1=0.0,
                                                        scalar2=None,
                                                        op0=OP.max)
                            else:
                                nc.scalar.activation(rt[:, i, :, :], vts[i],
                                                     AF.Relu)
                    for i in range(pn):
                        for k in range(kc):
                            nc.tensor.matmul(pa[:, i0 + i, :],
                                             lhsT=rt[:, i, k, :],
                                             rhs=ab_t[:, k, :], start=False,
                                             stop=(k == kc - 1))
                # ea = exp(alpha) for the whole quad -> msgs[:, :, w:w+H]
                nc.scalar.activation(msgs[:, :qn, w:w + H], pa[:, :qn, :],
                                     AF.Exp)
                # msgs[:, :, :w] = xl * ea (4D broadcast multiply)
                ch = w // H
                nc.vector.tensor_tensor(
                    out=_ap4(msgs[:], qn, H, ch, wt, ch, 1),
                    in0=_ap4(xl[:], qn, H, ch, wt, ch, 1),
                    in1=_ap4(msgs[:, 0, w:w + H], qn, H, ch, wt, 1, 0),
                    op=OP.mult)
                for i in range(qn):
                    last = (q == nq - 1) and (i == qn - 1)
                    if pd is None:
                        nc.tensor.matmul(pm[:], lhsT=oh[:, i, 0, :],
                                         rhs=msgs[:, i, :w + H], start=first,
                                         stop=last)
                    else:
                        nc.tensor.matmul(pm[:], lhsT=oh[:, i, 0, :],
                                         rhs=msgs[:, i, :w], start=first,
                                         stop=last)
                        nc.tensor.matmul(pd[:], lhsT=oh[:, i, 0, :],
                                         rhs=msgs[:, i, w:w + H], start=first,
                                         stop=last)
                    first = False

        def norm_heads(pm_ap, den_ap, w, tag):
            den = sb.tile([P, H], F32, tag="den")
            nc.vector.tensor_scalar(out=den[:], in0=den_ap, scalar1=1e-16,
                                    scalar2=None, op0=OP.add)
            rden = sb.tile([P, H], F32, tag="rden")
            nc.vector.reciprocal(rden[:], den[:])
            y = ev.tile([P, w], F32, tag=tag)
            ch = w // H
            nc.vector.tensor_tensor(
                out=_ap3(y[:], H, ch, ch, 1),
                in0=_ap3(pm_ap, H, ch, ch, 1),
                in1=_ap3(rden[:], H, ch, 1, 0),
                op=OP.mult)
            return y

        def elu(out_ap, y_ap, w):
            m = ev.tile([P, w], F32, tag="elu_m")
            nc.vector.tensor_scalar(out=m[:], in0=y_ap, scalar1=0.0,
                                    scalar2=None, op0=OP.min)
            e = ev.tile([P, w], F32, tag="elu_e")
            nc.scalar.activation(e[:], m[:], AF.Exp)
            r = ev.tile([P, w], F32, tag="elu_r")
            nc.scalar.activation(r[:], y_ap, AF.Relu)
            nc.vector.scalar_tensor_tensor(out=out_ap, in0=r[:], scalar=-1.0,
                                           in1=e[:], op0=OP.add, op1=OP.add)

        def dense_sb(x1T, W_t, n, bias_t, dst):
            """Dense from an SBUF bf16 lhsT, split to fit psum banks."""
            nmain = min(n, HC3)
            pt = pse.tile([P, nmain], F32, space="PSUM", tag="pm")
            for k in range(2):
                nc.tensor.matmul(pt[:], lhsT=x1T[:, k, :],
                                 rhs=W_t[:, k, :nmain], start=(k == 0),
                                 stop=(k == 1))
            o = sb.tile([P, n], BF16, tag="o3")
            if zb:
                nc.scalar.copy(o[:, :nmain], pt[:])
            else:
                nc.vector.tensor_tensor(out=o[:, :nmain], in0=pt[:],
                                        in1=bias_t[:, :nmain], op=OP.add)
            if n > nmain:
                px = psa.tile([P, n - nmain], F32, space="PSUM", tag="pa")
                for k in range(2):
                    nc.tensor.matmul(px[:], lhsT=x1T[:, k, :],
                                     rhs=W_t[:, k, nmain:n], start=(k == 0),
                                     stop=(k == 1))
                nc.vector.tensor_tensor(out=o[:, nmain:n], in0=px[:],
                                        in1=bias_t[:, nmain:n], op=OP.add)
            nc.sync.dma_start(dst, o[:])

        def transpose_to(x_ap, kc, tag):
            """fp32 x [128, kc*128] -> bf16 xT tiles [128, kc, 128]."""
            xT = sb.tile([P, kc, P], BF16, tag=tag)
            for k in range(kc):
                pt = psv.tile([P, 2, 2, P], F32, space="PSUM", tag="vt")
                nc.tensor.transpose(out=pt[:, 0, 0, :],
                                    in_=x_ap[:, k * P:(k + 1) * P],
                                    identity=ident32[:])
                nc.scalar.copy(xT[:, k, :], pt[:, 0, 0, :])
            return xT

        # ---- phase B: sg edges -> x1_gene -> agin_3, AG2
        ci = 0
        for slot in range(G_PER_CORE):
            xr_slot = xrs.tile([P, W1], BF16, tag="xr1")
            nc.sync.dma_start(xr_slot[:], xr1_sg[slot * P:(slot + 1) * P, :])
            pm = pse.tile([P, W1], F32, space="PSUM", tag="pm")
            nch = int(g_sched[slot])
            edge_slot(ci, nch, sg_srcr_t, sg_ohp, tbl_s, xr_slot, ab1_sg_t,
                      2, pm, None, HC1, W1)
            ci += nch
            y = norm_heads(pm[:, :HC1], pm[:, HC1:W1], HC1, "y1g")
            if zb:
                y2 = y
            else:
                y2 = ev.tile([P, HC1], F32, tag="y2g")
                nc.vector.tensor_tensor(out=y2[:], in0=y[:],
                                        in1=bias1_sg_t[:], op=OP.add)
            x1 = ev.tile([P, HC1], F32, tag="x1g")
            elu(x1[:], y2[:], HC1)
            x1T = transpose_to(x1[:], 2, "x1gT")
            dense_sb(x1T, Wl3_t, W3, bl3_t,
                     agin_3[slot * P:(slot + 1) * P, :])
        nc.gpsimd.collective_compute("AllGather", OP.bypass, replica_groups=RG,
                                     ins=[agin_3[:]], outs=[tbl_3[:]])

        # ---- phase C: gs edges -> x1_sample -> xr3/sl3 rows
        ci = 0
        for slot in range(S_PER_CORE):
            xr_slot = xrs.tile([P, W1], BF16, tag="xr1")
            nc.sync.dma_start(xr_slot[:], xr1_gs[slot * P:(slot + 1) * P, :])
            pm = pse.tile([P, W1], F32, space="PSUM", tag="pm")
            nch = int(s_sched[slot])
            edge_slot(ci, nch, gs_srcr_t, gs_ohp, tbl_g, xr_slot, ab1_gs_t,
                      2, pm, None, HC1, W1)
            ci += nch
            y = norm_heads(pm[:, :HC1], pm[:, HC1:W1], HC1, "y1s")
            if zb:
                y2 = y
            else:
                y2 = ev.tile([P, HC1], F32, tag="y2s")
                nc.vector.tensor_tensor(out=y2[:], in0=y[:],
                                        in1=bias1_gs_t[:], op=OP.add)
            y3 = ev.tile([P, HC1], F32, tag="y3s")
            sl1_ap = bass.AP(sl1_sb.tensor,
                             sl1_sb[:, slot * C1:(slot + 1) * C1].offset,
                             [[sl1_sb[:].ap[0][0], P], [0, H], [1, C1]])
            nc.vector.tensor_tensor(out=_ap3(y3[:], H, C1, C1, 1),
                                    in0=_ap3(y2[:], H, C1, C1, 1),
                                    in1=sl1_ap, op=OP.add)
            x1 = ev.tile([P, HC1], F32, tag="x1s")
            elu(x1[:], y3[:], HC1)
            x1T = transpose_to(x1[:], 2, "x1sT")
            dense_sb(x1T, Wr3_t, W3, br3_t, xr3[slot * P:(slot + 1) * P, :])
            pt2 = psa.tile([P, C3], F32, space="PSUM", tag="pa")
            for k in range(2):
                nc.tensor.matmul(pt2[:], lhsT=x1T[:, k, :], rhs=sl3_W_t[:, k, :],
                                 start=(k == 0), stop=(k == 1))
            if zb:
                nc.scalar.copy(sl3_sb[:, slot * C3:(slot + 1) * C3], pt2[:])
            else:
                nc.vector.tensor_tensor(
                    out=sl3_sb[:, slot * C3:(slot + 1) * C3],
                    in0=pt2[:], in1=sl3_b_t[:], op=OP.add)

        # ---- phase D: gs edges layer 3 -> output
        ci = 0
        for slot in range(S_PER_CORE):
            xr_slot = xrs.tile([P, W3], BF16, tag="xr3")
            nc.sync.dma_start(xr_slot[:], xr3[slot * P:(slot + 1) * P, :])
            pm = pse.tile([P, HC3], F32, space="PSUM", tag="pm")
            pd = psd.tile([P, H], F32, space="PSUM", tag="pd")
            nch = int(s_sched[slot])
            edge_slot(ci, nch, gs_srcr_t, gs_ohp, tbl_3, xr_slot, ab3_t,
                      4, pm, pd, HC3, W3)
            ci += nch
            den4 = sb.tile([P, H], F32, tag="den")
            nc.vector.tensor_scalar(out=den4[:], in0=pd[:], scalar1=4.0,
                                    scalar2=4e-16, op0=OP.mult, op1=OP.add)
            rden = sb.tile([P, H], F32, tag="rden")
            nc.vector.reciprocal(rden[:], den4[:])
            if zb:
                base = None
                accs = [sl3_sb[:, slot * C3:(slot + 1) * C3]]
            else:
                base = ev.tile([P, C3], F32, tag="based")
                nc.vector.tensor_tensor(
                    out=base[:], in0=sl3_sb[:, slot * C3:(slot + 1) * C3],
                    in1=bias3_t[:], op=OP.add)
                accs = [base[:]]
            for h in range(H):
                a = ev.tile([P, C3], F32, tag=f"acc{h}")
                nc.vector.scalar_tensor_tensor(
                    out=a[:], in0=pm[:, h * C3:(h + 1) * C3],
                    scalar=rden[:, h:h + 1], in1=accs[-1],
                    op0=OP.mult, op1=OP.add)
                accs.append(a[:])
            o = ev.tile([P, C3], F32, tag="outt")
            elu(o[:], accs[-1], C3)
            nc.sync.dma_start(out_own[slot * P:(slot + 1) * P, :], o[:])

    nc.compile()
    return nc


# ------------------------------------------------------------------ driver

_CACHE = {}


def kernel(**inputs):
    plan, in_maps = _plan(inputs)
    key = (tuple(plan["g_sched"]), tuple(plan["s_sched"]), plan["zb"])
    if key not in _CACHE:
        _CACHE[key] = _build(plan["g_sched"], plan["s_sched"], plan["zb"])
    nc = _CACHE[key]
    r = run_bass_kernel_spmd(nc, in_maps, core_ids=list(range(NCORES)))
    out = np.zeros((NS, C3), np.float32)
    s_assign = plan["s_assign"]
    for c in range(NCORES):
        oc = r.results[c]["out_own"]
        for slot in range(S_PER_CORE):
            t = s_assign[slot, c]
            out[t * P:(t + 1) * P] = oc[slot * P:(slot + 1) * P]
    return out


# revision 18
# speedup vs baseline: 1.0241x; 1.0241x over previous
"""BiRGAT (bipartite 2-layer GATv2) Trainium2 kernel, 8-core SPMD.

Destination-tile sharding (as v1) with a restructured bf16 edge phase:
- Tables (layer-1/3 transforms) stored bf16 with 4 appended columns holding
  0.2*(att . row) partial dots, so the leaky-relu attention score splits as
  alpha = 0.8*(att . relu(v)) + 0.2*(att . v) with the linear part gathered
  for free (lrelu(v) = 0.2 v + 0.8 relu(v)).
- One-hot scatter matrices (onehot [e,d] and its transpose [d,e]) are
  precomputed on host per chunk and DMA'd (bf16), removing the per-chunk
  iota/is_equal/PE-transpose/PSUM-copy chain.
- Per chunk: indirect-DMA gather of 260/516-wide source rows; v^T built on
  PE via transpose-matmul (rhs=identity) + xr scatter-matmul (rhs=onehotT)
  into PSUM; relu (Act/DVE alternating) -> rT bf16; alpha on PE
  (lhsT=rT k-tiles, rhs=0.8-scaled block-diagonal att + linear-part mms);
  exp batched over 4 chunks into msgs[:, w:w+4]; one 4D DVE multiply forms
  msgs; one-hot scatter matmul accumulates [messages | ea] per slot in PSUM.
- Softmax normalization per slot in fp32 (no segment_max; alpha std ~0.3).
"""
import sys

sys.path.insert(0, "/opt/trn_rl_repo")

import numpy as np
import ml_dtypes
from contextlib import ExitStack

import concourse.bass as bass
import concourse.tile as tile
from concourse import bacc, mybir
from concourse.bass_utils import run_bass_kernel_spmd
from concourse.masks import make_identity

P = 128
NCORES = 8
NS, NG, E = 4096, 20000, 131072
DIN, H, C1, C3 = 256, 4, 64, 128
HC1, HC3 = H * C1, H * C3          # 256, 512
W1, W3 = HC1 + H, HC3 + H         # 260, 516 (table row widths)
NGP = 20480                        # genes padded to 160 tiles
NST, NGT = NS // P, NGP // P       # 32, 160
S_PER_CORE, G_PER_CORE = NST // NCORES, NGT // NCORES   # 4, 20
SROWS, GROWS = S_PER_CORE * P, G_PER_CORE * P           # 512, 2560

F32 = mybir.dt.float32
BF16 = mybir.dt.bfloat16
I32 = mybir.dt.int32
AF = mybir.ActivationFunctionType
OP = mybir.AluOpType
BF = ml_dtypes.bfloat16


# ---------------------------------------------------------------- host plan

def _deal_tiles(dst, n_tiles, per_core):
    tcnt = np.bincount(dst // P, minlength=n_tiles)
    chunks = np.maximum((tcnt + P - 1) // P, 1)
    order = np.argsort(-chunks, kind="stable")
    assign = order.reshape(per_core, NCORES)
    sched = chunks[assign].max(axis=1)
    return assign, sched.astype(int)


def _edge_arrays(src, dst, assign, sched, src_row_map, core):
    """srcr [128, nch] i32 table-row per edge; ohp [128, nch, 2, 128] bf16
    with [:, c, 0, :] = onehot[e, d] and [:, c, 1, :] = onehotT[d, e]."""
    nch = int(sched.sum())
    srcr = np.zeros((P, nch), np.int32)
    ohp = np.zeros((P, nch, 2, P), BF)
    tile_of = dst // P
    ci = 0
    for slot in range(len(sched)):
        t = assign[slot, core]
        e = np.nonzero(tile_of == t)[0]
        n = len(e)
        rows = src_row_map[src[e]]
        dl = (dst[e] % P).astype(np.int64)
        for k in range(int(sched[slot])):
            lo, hi = k * P, min((k + 1) * P, n)
            cnt = hi - lo
            if cnt > 0:
                srcr[:cnt, ci] = rows[lo:hi]
                ohp[np.arange(cnt), ci, 0, dl[lo:hi]] = 1.0
                ohp[dl[lo:hi], ci, 1, np.arange(cnt)] = 1.0
            ci += 1
    return srcr, ohp


def _rhs(w):
    """[K, N] -> [128, K//128, N] rhs tile layout."""
    K, N = w.shape
    return np.ascontiguousarray(
        w.reshape(K // P, P, N).transpose(1, 0, 2)).astype(BF)


def _xt(x):
    """[R, K] -> [128, K//128, R] lhsT layout (features on partitions)."""
    R, K = x.shape
    return np.ascontiguousarray(
        x.T.reshape(K // P, P, R).transpose(1, 0, 2)).astype(BF)


def _ext_w(Wl, bl, att, out_c):
    """Extend [K, H*out_c] weight with 4 cols of 0.2*(att . W) partial dots.
    Returns (W_ext [K, H*out_c+4] f32, b_ext [H*out_c+4] f32)."""
    K = Wl.shape[0]
    Wp = np.einsum('khc,hc->kh', Wl.reshape(K, H, out_c), att) * 0.2
    bp = np.einsum('hc,hc->h', bl.reshape(H, out_c), att) * 0.2
    return (np.concatenate([Wl, Wp], axis=1),
            np.concatenate([bl, bp], axis=0))


def _attblk(att, out_c, kc):
    """[128, kc, H] bf16 block-diagonal attention, scaled by 0.8."""
    a = np.zeros((P, kc, H), np.float32)
    for k in range(kc):
        for p in range(P):
            f = k * P + p
            a[p, k, f // out_c] = 0.8 * att[f // out_c, f % out_c]
    return a.astype(BF)


def _bcast(v):
    v = np.asarray(v, np.float32).reshape(-1)
    return np.broadcast_to(v[None, :], (P, len(v))).copy()


def _plan(inputs):
    f32 = lambda k: np.asarray(inputs[k], np.float32)
    sg_src = np.asarray(inputs["sg_src"]); sg_dst = np.asarray(inputs["sg_dst"])
    gs_src = np.asarray(inputs["gs_src"]); gs_dst = np.asarray(inputs["gs_dst"])

    g_assign, g_sched = _deal_tiles(sg_dst, NGT, G_PER_CORE)
    s_assign, s_sched = _deal_tiles(gs_dst, NST, S_PER_CORE)

    g_owner = np.zeros(NGT, np.int64); g_slot = np.zeros(NGT, np.int64)
    for slot in range(G_PER_CORE):
        for c in range(NCORES):
            g_owner[g_assign[slot, c]] = c
            g_slot[g_assign[slot, c]] = slot
    s_owner = np.zeros(NST, np.int64); s_slot = np.zeros(NST, np.int64)
    for slot in range(S_PER_CORE):
        for c in range(NCORES):
            s_owner[s_assign[slot, c]] = c
            s_slot[s_assign[slot, c]] = slot

    sid = np.arange(NS)
    srow_tbl = s_owner[sid // P] * SROWS + s_slot[sid // P] * P + sid % P
    gid = np.arange(NG)
    grow_tbl = g_owner[gid // P] * GROWS + g_slot[gid // P] * P + gid % P

    att1_sg = f32("att1_sg"); att1_gs = f32("att1_gs"); att3 = f32("att3_gs")

    Wl1_sg_e, bl1_sg_e = _ext_w(f32("Wl1_sg"), f32("bl1_sg"), att1_sg, C1)
    Wr1_sg_e, br1_sg_e = _ext_w(f32("Wr1_sg"), f32("br1_sg"), att1_sg, C1)
    Wl1_gs_e, bl1_gs_e = _ext_w(f32("Wl1_gs"), f32("bl1_gs"), att1_gs, C1)
    Wr1_gs_e, br1_gs_e = _ext_w(f32("Wr1_gs"), f32("br1_gs"), att1_gs, C1)
    Wl3_e, bl3_e = _ext_w(f32("Wl3_gs"), f32("bl3_gs"), att3, C3)
    Wr3_e, br3_e = _ext_w(f32("Wr3_gs"), f32("br3_gs"), att3, C3)

    x_sample = f32("x_sample"); x_gene = f32("x_gene")

    in_maps = []
    for c in range(NCORES):
        xs_own = x_sample.reshape(NST, P, DIN)[s_assign[:, c]].reshape(SROWS, DIN)
        xg_own = np.zeros((GROWS, DIN), np.float32)
        for i, t in enumerate(g_assign[:, c]):
            lo = t * P
            if lo < NG:
                n = min(P, NG - lo)
                xg_own[i * P:i * P + n] = x_gene[lo:lo + n]

        sg_srcr, sg_ohp = _edge_arrays(sg_src, sg_dst, g_assign, g_sched,
                                       srow_tbl, c)
        gs_srcr, gs_ohp = _edge_arrays(gs_src, gs_dst, s_assign, s_sched,
                                       grow_tbl, c)

        m = {
            "xsT": _xt(xs_own), "xgT": _xt(xg_own),
            "Wl1_sg": _rhs(Wl1_sg_e), "Wr1_sg": _rhs(Wr1_sg_e),
            "Wl1_gs": _rhs(Wl1_gs_e), "Wr1_gs": _rhs(Wr1_gs_e),
            "Wl3": _rhs(Wl3_e), "Wr3": _rhs(Wr3_e),
            "sl1_W": _rhs(f32("sl1_W")), "sl3_W": _rhs(f32("sl3_W")),
            "bl1_sg_b": _bcast(bl1_sg_e).astype(BF),
            "br1_sg_b": _bcast(br1_sg_e).astype(BF),
            "bl1_gs_b": _bcast(bl1_gs_e).astype(BF),
            "br1_gs_b": _bcast(br1_gs_e).astype(BF),
            "bl3_b": _bcast(bl3_e).astype(BF),
            "br3_b": _bcast(br3_e).astype(BF),
            "ab1_sg": _attblk(att1_sg, C1, 2),
            "ab1_gs": _attblk(att1_gs, C1, 2),
            "ab3": _attblk(att3, C3, 4),
            "bias1_sg_b": _bcast(inputs["bias1_sg"]),
            "bias1_gs_b": _bcast(inputs["bias1_gs"]),
            "bias3_b": _bcast(inputs["bias3_gs"]),
            "sl1_b_b": _bcast(inputs["sl1_b"]),
            "sl3_b_b": _bcast(inputs["sl3_b"]),
            "identb": np.eye(P, dtype=BF),
            "sg_srcr": sg_srcr, "gs_srcr": gs_srcr,
            "sg_ohp": sg_ohp, "gs_ohp": gs_ohp,
        }
        in_maps.append(m)
    zb = not any(np.any(np.asarray(inputs[k])) for k in
                 ("bl1_sg", "br1_sg", "bl1_gs", "br1_gs", "bl3_gs", "br3_gs",
                  "bias1_sg", "bias1_gs", "bias3_gs", "sl1_b", "sl3_b"))
    plan = {"g_sched": g_sched, "s_sched": s_sched, "s_assign": s_assign,
            "zb": zb}
    return plan, in_maps


# ------------------------------------------------------------- device build

def _ap3(base_ap, h, c, mid, inner):
    return bass.AP(base_ap.tensor, base_ap.offset,
                   [[base_ap.ap[0][0], P], [mid, h], [inner, c]])


def _ap4(base_ap, n, h, c, d0, mid, inner):
    return bass.AP(base_ap.tensor, base_ap.offset,
                   [[base_ap.ap[0][0], P], [d0, n], [mid, h], [inner, c]])


def _build(g_sched, s_sched, zb):
    nsg = int(g_sched.sum())
    ngs = int(s_sched.sum())
    nc = bacc.Bacc("TRN2", target_bir_lowering=False, debug=False,
                   num_devices=NCORES)

    ei = lambda name, shape, dt=BF16: nc.dram_tensor(name, shape, dt,
                                                     kind="ExternalInput")
    xsT = ei("xsT", [P, 2, SROWS]); xgT = ei("xgT", [P, 2, GROWS])
    Wl1_sg = ei("Wl1_sg", [P, 2, W1]); Wr1_sg = ei("Wr1_sg", [P, 2, W1])
    Wl1_gs = ei("Wl1_gs", [P, 2, W1]); Wr1_gs = ei("Wr1_gs", [P, 2, W1])
    Wl3 = ei("Wl3", [P, 2, W3]); Wr3 = ei("Wr3", [P, 2, W3])
    sl1_W = ei("sl1_W", [P, 2, C1]); sl3_W = ei("sl3_W", [P, 2, C3])
    bl1_sg_b = ei("bl1_sg_b", [P, W1]); br1_sg_b = ei("br1_sg_b", [P, W1])
    bl1_gs_b = ei("bl1_gs_b", [P, W1]); br1_gs_b = ei("br1_gs_b", [P, W1])
    bl3_b = ei("bl3_b", [P, W3]); br3_b = ei("br3_b", [P, W3])
    ab1_sg = ei("ab1_sg", [P, 2, H]); ab1_gs = ei("ab1_gs", [P, 2, H])
    ab3 = ei("ab3", [P, 4, H])
    bias1_sg_b = ei("bias1_sg_b", [P, HC1], F32)
    bias1_gs_b = ei("bias1_gs_b", [P, HC1], F32)
    bias3_b = ei("bias3_b", [P, C3], F32)
    sl1_b_b = ei("sl1_b_b", [P, C1], F32)
    sl3_b_b = ei("sl3_b_b", [P, C3], F32)
    identb = ei("identb", [P, P])
    sg_srcr = ei("sg_srcr", [P, nsg], I32)
    gs_srcr = ei("gs_srcr", [P, ngs], I32)
    sg_ohp = ei("sg_ohp", [P, nsg, 2, P])
    gs_ohp = ei("gs_ohp", [P, ngs, 2, P])

    out_own = nc.dram_tensor("out_own", [SROWS, C3], F32, kind="ExternalOutput")

    agin_s = nc.dram_tensor("agin_s", [SROWS, W1], BF16)
    agin_g = nc.dram_tensor("agin_g", [GROWS, W1], BF16)
    agin_3 = nc.dram_tensor("agin_3", [GROWS, W3], BF16)
    tbl_s = nc.dram_tensor("tbl_s", [NS, W1], BF16, addr_space="Shared")
    tbl_g = nc.dram_tensor("tbl_g", [NGP, W1], BF16, addr_space="Shared")
    tbl_3 = nc.dram_tensor("tbl_3", [NGP, W3], BF16, addr_space="Shared")
    xr1_sg = nc.dram_tensor("xr1_sg", [GROWS, W1], BF16)
    xr1_gs = nc.dram_tensor("xr1_gs", [SROWS, W1], BF16)
    xr3 = nc.dram_tensor("xr3", [SROWS, W3], BF16)

    RG = [list(range(NCORES))]

    with tile.TileContext(nc) as tc, ExitStack() as ctx:
        res = ctx.enter_context(tc.tile_pool(name="res", bufs=1))
        sb = ctx.enter_context(tc.tile_pool(name="sb", bufs=4))
        ev = ctx.enter_context(tc.tile_pool(name="ev", bufs=3))
        xlp = ctx.enter_context(tc.tile_pool(name="xlp", bufs=8))
        ohpool = ctx.enter_context(tc.tile_pool(name="ohpool", bufs=8))
        msp = ctx.enter_context(tc.tile_pool(name="msp", bufs=6))
        rtp = ctx.enter_context(tc.tile_pool(name="rtp", bufs=8))
        xrs = ctx.enter_context(tc.tile_pool(name="xrs", bufs=3))
        psv = ctx.enter_context(tc.tile_pool(name="psv", bufs=3, space="PSUM"))
        psa = ctx.enter_context(tc.tile_pool(name="psa", bufs=2, space="PSUM"))
        pse = ctx.enter_context(tc.tile_pool(name="pse", bufs=2, space="PSUM"))
        psd = ctx.enter_context(tc.tile_pool(name="psd", bufs=1, space="PSUM"))

        def rload(name, dram, shape, dt=BF16):
            t = res.tile(shape, dt, tag=name)
            nc.sync.dma_start(t[:], dram[:])
            return t

        identb_t = rload("identb", identb, [P, P])
        ident32 = res.tile([P, P], F32, tag="ident32")
        make_identity(nc, ident32[:])
        xsT_t = rload("xsT", xsT, [P, 2, SROWS])
        xgT_t = rload("xgT", xgT, [P, 2, GROWS])
        Wl1_sg_t = rload("Wl1_sg", Wl1_sg, [P, 2, W1])
        Wr1_sg_t = rload("Wr1_sg", Wr1_sg, [P, 2, W1])
        Wl1_gs_t = rload("Wl1_gs", Wl1_gs, [P, 2, W1])
        Wr1_gs_t = rload("Wr1_gs", Wr1_gs, [P, 2, W1])
        Wl3_t = rload("Wl3", Wl3, [P, 2, W3])
        Wr3_t = rload("Wr3", Wr3, [P, 2, W3])
        sl1_W_t = rload("sl1_W", sl1_W, [P, 2, C1])
        sl3_W_t = rload("sl3_W", sl3_W, [P, 2, C3])
        bl1_sg_t = rload("bl1_sg", bl1_sg_b, [P, W1])
        br1_sg_t = rload("br1_sg", br1_sg_b, [P, W1])
        bl1_gs_t = rload("bl1_gs", bl1_gs_b, [P, W1])
        br1_gs_t = rload("br1_gs", br1_gs_b, [P, W1])
        bl3_t = rload("bl3", bl3_b, [P, W3])
        br3_t = rload("br3", br3_b, [P, W3])
        ab1_sg_t = rload("ab1_sg", ab1_sg, [P, 2, H])
        ab1_gs_t = rload("ab1_gs", ab1_gs, [P, 2, H])
        ab3_t = rload("ab3", ab3, [P, 4, H])
        bias1_sg_t = rload("bias1_sg", bias1_sg_b, [P, HC1], F32)
        bias1_gs_t = rload("bias1_gs", bias1_gs_b, [P, HC1], F32)
        bias3_t = rload("bias3", bias3_b, [P, C3], F32)
        sl1_b_t = rload("sl1_b", sl1_b_b, [P, C1], F32)
        sl3_b_t = rload("sl3_b", sl3_b_b, [P, C3], F32)
        sg_srcr_t = rload("sg_srcr", sg_srcr, [P, nsg], I32)
        gs_srcr_t = rload("gs_srcr", gs_srcr, [P, ngs], I32)

        sl1_sb = res.tile([P, S_PER_CORE * C1], F32, tag="sl1_sb")
        sl3_sb = res.tile([P, S_PER_CORE * C3], F32, tag="sl3_sb")

        # ---- phase A: node tables (xT shipped pre-transposed from host)
        def dense(xT_t, lo, W_t, n, bias_t, dst, tag):
            """n <= 512: single-bank psum; n > 512: split main/extra cols."""
            nmain = min(n, HC3)
            pt = pse.tile([P, nmain], F32, space="PSUM", tag="pm")
            for k in range(2):
                nc.tensor.matmul(pt[:], lhsT=xT_t[:, k, lo:lo + P],
                                 rhs=W_t[:, k, :nmain], start=(k == 0),
                                 stop=(k == 1))
            o = sb.tile([P, n], BF16, tag=tag)
            nc.vector.tensor_tensor(out=o[:, :nmain], in0=pt[:],
                                    in1=bias_t[:, :nmain], op=OP.add)
            if n > nmain:
                px = psa.tile([P, n - nmain], F32, space="PSUM", tag="pa")
                for k in range(2):
                    nc.tensor.matmul(px[:], lhsT=xT_t[:, k, lo:lo + P],
                                     rhs=W_t[:, k, nmain:n], start=(k == 0),
                                     stop=(k == 1))
                nc.vector.tensor_tensor(out=o[:, nmain:n], in0=px[:],
                                        in1=bias_t[:, nmain:n], op=OP.add)
            nc.sync.dma_start(dst, o[:])

        for i in range(S_PER_CORE):
            lo = i * P
            dense(xsT_t, lo, Wl1_sg_t, W1, bl1_sg_t,
                  agin_s[lo:lo + P, :], "da")
            dense(xsT_t, lo, Wr1_gs_t, W1, br1_gs_t,
                  xr1_gs[lo:lo + P, :], "dx")
            pt = psa.tile([P, C1], F32, space="PSUM", tag="pa")
            for k in range(2):
                nc.tensor.matmul(pt[:], lhsT=xsT_t[:, k, lo:lo + P],
                                 rhs=sl1_W_t[:, k, :], start=(k == 0),
                                 stop=(k == 1))
            if zb:
                nc.scalar.copy(sl1_sb[:, i * C1:(i + 1) * C1], pt[:])
            else:
                nc.vector.tensor_tensor(out=sl1_sb[:, i * C1:(i + 1) * C1],
                                        in0=pt[:], in1=sl1_b_t[:], op=OP.add)
        nc.gpsimd.collective_compute("AllGather", OP.bypass, replica_groups=RG,
                                     ins=[agin_s[:]], outs=[tbl_s[:]])

        for j in range(G_PER_CORE):
            lo = j * P
            dense(xgT_t, lo, Wl1_gs_t, W1, bl1_gs_t,
                  agin_g[lo:lo + P, :], "da")
            dense(xgT_t, lo, Wr1_sg_t, W1, br1_sg_t,
                  xr1_sg[lo:lo + P, :], "dx")
        nc.gpsimd.collective_compute("AllGather", OP.bypass, replica_groups=RG,
                                     ins=[agin_g[:]], outs=[tbl_g[:]])

        # ---- edge phase helper -------------------------------------------
        relu_ctr = [0]

        def edge_slot(ci0, nch, srcr_t, ohp_dram, tbl, xr_slot, ab_t, kc,
                      pm, pd, w, wt):
            """Process chunks ci0..ci0+nch-1 of one slot.
            kc: feature k-tiles (2 or 4); w: message width (256/512);
            wt: table row width (260/516). pm [P, w(+H)] psum accumulates
            messages (+ea for w=256); pd [P, H] psum for w=512."""
            first = True
            nq = (nch + 3) // 4
            for q in range(nq):
                qn = min(4, nch - q * 4)
                cq = ci0 + q * 4
                # batched onehot-pair load (SP queue)
                oh = ohpool.tile([P, 4, 2, P], BF16, tag="oh")
                nc.sync.dma_start(oh[:, :qn, :, :],
                                  ohp_dram[:, cq:cq + qn, :, :])
                xl = xlp.tile([P, 4, wt], BF16, tag=f"xl{wt}")
                for i in range(qn):
                    nc.gpsimd.indirect_dma_start(
                        out=xl[:, i, :], out_offset=None, in_=tbl[:],
                        in_offset=bass.IndirectOffsetOnAxis(
                            ap=srcr_t[:, cq + i:cq + i + 1], axis=0))
                pa = psa.tile([P, 4, H], F32, space="PSUM", tag="pa")
                msgs = msp.tile([P, 4, wt], BF16, tag=f"ms{wt}")
                for i0 in range(0, qn, 2):
                    pn = min(2, qn - i0)
                    if w == HC1:
                        vt = psv.tile([P, 2, kc, P], F32, space="PSUM",
                                      tag="vt")
                        vts = [vt[:, i, :, :] for i in range(pn)]
                    else:
                        vts = []
                        for i in range(pn):
                            v = psv.tile([P, 1, kc, P], F32, space="PSUM",
                                         tag="vt")
                            vts.append(v[:, 0, :, :])
                    for i in range(pn):
                        c = i0 + i
                        # vT = xl^T + onehotT @ xr (kc tiles, one psum bank)
                        for k in range(kc):
                            nc.tensor.matmul(vts[i][:, k, :],
                                             lhsT=xl[:, c, k * P:(k + 1) * P],
                                             rhs=identb_t[:], start=True,
                                             stop=False)
                            nc.tensor.matmul(vts[i][:, k, :],
                                             lhsT=xr_slot[:, k * P:(k + 1) * P],
                                             rhs=oh[:, c, 1, :], start=False,
                                             stop=True)
                    rt = rtp.tile([P, 2, kc, P], BF16, tag="rt")
                    rc = relu_ctr[0]; relu_ctr[0] += 1
                    if w == HC1 and pn == 2:
                        if rc % 5 < 1:
                            nc.vector.tensor_scalar(out=rt[:], in0=vt[:],
                                                    scalar1=0.0, scalar2=None,
                                                    op0=OP.max)
                        else:
                            nc.scalar.activation(rt[:], vt[:], AF.Relu)
                    else:
                        for i in range(pn):
                            if rc % 5 < 1:
                                nc.vector.tensor_scalar(out=rt[:, i, :, :],
                                                        in0=vts[i],
                                                        scalar1=0.0,
                                                        scalar2=None,
                                                        op0=OP.max)
                            else:
                                nc.scalar.activation(rt[:, i, :, :], vts[i],
                                                     AF.Relu)
                    for i in range(pn):
                        c = i0 + i
                        # alpha: linear part (0.2-scaled columns), late so
                        # the pa psum tile is held briefly
                        nc.tensor.matmul(pa[:, c, :], lhsT=identb_t[:],
                                         rhs=xl[:, c, w:w + H], start=True,
                                         stop=False)
                        nc.tensor.matmul(pa[:, c, :], lhsT=oh[:, c, 1, :],
                                         rhs=xr_slot[:, w:w + H], start=False,
                                         stop=False)
                        for k in range(kc):
                            nc.tensor.matmul(pa[:, c, :],
                                             lhsT=rt[:, i, k, :],
                                             rhs=ab_t[:, k, :], start=False,
                                             stop=(k == kc - 1))
                # ea = exp(alpha) for the whole quad -> msgs[:, :, w:w+H]
                nc.scalar.activation(msgs[:, :qn, w:w + H], pa[:, :qn, :],
                                     AF.Exp)
                # msgs[:, :, :w] = xl * ea (4D broadcast multiply)
                ch = w // H
                nc.vector.tensor_tensor(
                    out=_ap4(msgs[:], qn, H, ch, wt, ch, 1),
                    in0=_ap4(xl[:], qn, H, ch, wt, ch, 1),
                    in1=_ap4(msgs[:, 0, w:w + H], qn, H, ch, wt, 1, 0),
                    op=OP.mult)
                for i in range(qn):
                    last = (q == nq - 1) and (i == qn - 1)
                    if pd is None:
                        nc.tensor.matmul(pm[:], lhsT=oh[:, i, 0, :],
                                         rhs=msgs[:, i, :w + H], start=first,
                                         stop=last)
                    else:
                        nc.tensor.matmul(pm[:], lhsT=oh[:, i, 0, :],
                                         rhs=msgs[:, i, :w], start=first,
                                         stop=last)
                        nc.tensor.matmul(pd[:], lhsT=oh[:, i, 0, :],
                                         rhs=msgs[:, i, w:w + H], start=first,
                                         stop=last)
                    first = False

        def norm_heads(pm_ap, den_ap, w, tag):
            den = sb.tile([P, H], F32, tag="den")
            nc.vector.tensor_scalar(out=den[:], in0=den_ap, scalar1=1e-16,
                                    scalar2=None, op0=OP.add)
            rden = sb.tile([P, H], F32, tag="rden")
            nc.vector.reciprocal(rden[:], den[:])
            y = ev.tile([P, w], F32, tag=tag)
            ch = w // H
            nc.vector.tensor_tensor(
                out=_ap3(y[:], H, ch, ch, 1),
                in0=_ap3(pm_ap, H, ch, ch, 1),
                in1=_ap3(rden[:], H, ch, 1, 0),
                op=OP.mult)
            return y

        def elu(out_ap, y_ap, w):
            m = ev.tile([P, w], F32, tag="elu_m")
            nc.vector.tensor_scalar(out=m[:], in0=y_ap, scalar1=0.0,
                                    scalar2=None, op0=OP.min)
            e = ev.tile([P, w], F32, tag="elu_e")
            nc.scalar.activation(e[:], m[:], AF.Exp)
            r = ev.tile([P, w], F32, tag="elu_r")
            nc.scalar.activation(r[:], y_ap, AF.Relu)
            nc.vector.scalar_tensor_tensor(out=out_ap, in0=r[:], scalar=-1.0,
                                           in1=e[:], op0=OP.add, op1=OP.add)

        def dense_sb(x1T, W_t, n, bias_t, dst):
            """Dense from an SBUF bf16 lhsT, split to fit psum banks."""
            nmain = min(n, HC3)
            pt = pse.tile([P, nmain], F32, space="PSUM", tag="pm")
            for k in range(2):
                nc.tensor.matmul(pt[:], lhsT=x1T[:, k, :],
                                 rhs=W_t[:, k, :nmain], start=(k == 0),
                                 stop=(k == 1))
            o = sb.tile([P, n], BF16, tag="o3")
            if zb:
                nc.scalar.copy(o[:, :nmain], pt[:])
            else:
                nc.vector.tensor_tensor(out=o[:, :nmain], in0=pt[:],
                                        in1=bias_t[:, :nmain], op=OP.add)
            if n > nmain:
                px = psa.tile([P, n - nmain], F32, space="PSUM", tag="pa")
                for k in range(2):
                    nc.tensor.matmul(px[:], lhsT=x1T[:, k, :],
                                     rhs=W_t[:, k, nmain:n], start=(k == 0),
                                     stop=(k == 1))
                nc.vector.tensor_tensor(out=o[:, nmain:n], in0=px[:],
                                        in1=bias_t[:, nmain:n], op=OP.add)
            nc.sync.dma_start(dst, o[:])

        def transpose_to(x_ap, kc, tag):
            """fp32 x [128, kc*128] -> bf16 xT tiles [128, kc, 128]."""
            xT = sb.tile([P, kc, P], BF16, tag=tag)
            for k in range(kc):
                pt = psv.tile([P, 2, 2, P], F32, space="PSUM", tag="vt")
                nc.tensor.transpose(out=pt[:, 0, 0, :],
                                    in_=x_ap[:, k * P:(k + 1) * P],
                                    identity=ident32[:])
                nc.scalar.copy(xT[:, k, :], pt[:, 0, 0, :])
            return xT

        # ---- phase B: sg edges -> x1_gene -> agin_3, AG2
        ci = 0
        for slot in range(G_PER_CORE):
            xr_slot = xrs.tile([P, W1], BF16, tag="xr1")
            nc.sync.dma_start(xr_slot[:], xr1_sg[slot * P:(slot + 1) * P, :])
            pm = pse.tile([P, W1], F32, space="PSUM", tag="pm")
            nch = int(g_sched[slot])
            edge_slot(ci, nch, sg_srcr_t, sg_ohp, tbl_s, xr_slot, ab1_sg_t,
                      2, pm, None, HC1, W1)
            ci += nch
            y = norm_heads(pm[:, :HC1], pm[:, HC1:W1], HC1, "y1g")
            if zb:
                y2 = y
            else:
                y2 = ev.tile([P, HC1], F32, tag="y2g")
                nc.vector.tensor_tensor(out=y2[:], in0=y[:],
                                        in1=bias1_sg_t[:], op=OP.add)
            x1 = ev.tile([P, HC1], F32, tag="x1g")
            elu(x1[:], y2[:], HC1)
            x1T = transpose_to(x1[:], 2, "x1gT")
            dense_sb(x1T, Wl3_t, W3, bl3_t,
                     agin_3[slot * P:(slot + 1) * P, :])
        nc.gpsimd.collective_compute("AllGather", OP.bypass, replica_groups=RG,
                                     ins=[agin_3[:]], outs=[tbl_3[:]])

        # ---- phase C: gs edges -> x1_sample -> xr3/sl3 rows
        ci = 0
        for slot in range(S_PER_CORE):
            xr_slot = xrs.tile([P, W1], BF16, tag="xr1")
            nc.sync.dma_start(xr_slot[:], xr1_gs[slot * P:(slot + 1) * P, :])
            pm = pse.tile([P, W1], F32, space="PSUM", tag="pm")
            nch = int(s_sched[slot])
            edge_slot(ci, nch, gs_srcr_t, gs_ohp, tbl_g, xr_slot, ab1_gs_t,
                      2, pm, None, HC1, W1)
            ci += nch
            y = norm_heads(pm[:, :HC1], pm[:, HC1:W1], HC1, "y1s")
            if zb:
                y2 = y
            else:
                y2 = ev.tile([P, HC1], F32, tag="y2s")
                nc.vector.tensor_tensor(out=y2[:], in0=y[:],
                                        in1=bias1_gs_t[:], op=OP.add)
            y3 = ev.tile([P, HC1], F32, tag="y3s")
            sl1_ap = bass.AP(sl1_sb.tensor,
                             sl1_sb[:, slot * C1:(slot + 1) * C1].offset,
                             [[sl1_sb[:].ap[0][0], P], [0, H], [1, C1]])
            nc.vector.tensor_tensor(out=_ap3(y3[:], H, C1, C1, 1),
                                    in0=_ap3(y2[:], H, C1, C1, 1),
                                    in1=sl1_ap, op=OP.add)
            x1 = ev.tile([P, HC1], F32, tag="x1s")
            elu(x1[:], y3[:], HC1)
            x1T = transpose_to(x1[:], 2, "x1sT")
            dense_sb(x1T, Wr3_t, W3, br3_t, xr3[slot * P:(slot + 1) * P, :])
            pt2 = psa.tile([P, C3], F32, space="PSUM", tag="pa")
            for k in range(2):
                nc.tensor.matmul(pt2[:], lhsT=x1T[:, k, :], rhs=sl3_W_t[:, k, :],
                                 start=(k == 0), stop=(k == 1))
            if zb:
                nc.scalar.copy(sl3_sb[:, slot * C3:(slot + 1) * C3], pt2[:])
            else:
                nc.vector.tensor_tensor(
                    out=sl3_sb[:, slot * C3:(slot + 1) * C3],
                    in0=pt2[:], in1=sl3_b_t[:], op=OP.add)

        # ---- phase D: gs edges layer 3 -> output
        ci = 0
        for slot in range(S_PER_CORE):
            xr_slot = xrs.tile([P, W3], BF16, tag="xr3")
            nc.sync.dma_start(xr_slot[:], xr3[slot * P:(slot + 1) * P, :])
            pm = pse.tile([P, HC3], F32, space="PSUM", tag="pm")
            pd = psd.tile([P, H], F32, space="PSUM", tag="pd")
            nch = int(s_sched[slot])
            edge_slot(ci, nch, gs_srcr_t, gs_ohp, tbl_3, xr_slot, ab3_t,
                      4, pm, pd, HC3, W3)
            ci += nch
            den4 = sb.tile([P, H], F32, tag="den")
            nc.vector.tensor_scalar(out=den4[:], in0=pd[:], scalar1=4.0,
                                    scalar2=4e-16, op0=OP.mult, op1=OP.add)
            rden = sb.tile([P, H], F32, tag="rden")
            nc.vector.reciprocal(rden[:], den4[:])
            if zb:
                base = None
                accs = [sl3_sb[:, slot * C3:(slot + 1) * C3]]
            else:
                base = ev.tile([P, C3], F32, tag="based")
                nc.vector.tensor_tensor(
                    out=base[:], in0=sl3_sb[:, slot * C3:(slot + 1) * C3],
                    in1=bias3_t[:], op=OP.add)
                accs = [base[:]]
            for h in range(H):
                a = ev.tile([P, C3], F32, tag=f"acc{h}")
                nc.vector.scalar_tensor_tensor(
                    out=a[:], in0=pm[:, h * C3:(h + 1) * C3],
                    scalar=rden[:, h:h + 1], in1=accs[-1],
                    op0=OP.mult, op1=OP.add)
                accs.append(a[:])
            o = ev.tile([P, C3], F32, tag="outt")
            elu(o[:], accs[-1], C3)
            nc.sync.dma_start(out_own[slot * P:(slot + 1) * P, :], o[:])

    nc.compile()
    return nc


# ------------------------------------------------------------------ driver

_CACHE = {}


def kernel(**inputs):
    plan, in_maps = _plan(inputs)
    key = (tuple(plan["g_sched"]), tuple(plan["s_sched"]), plan["zb"])
    if key not in _CACHE:
        _CACHE[key] = _build(plan["g_sched"], plan["s_sched"], plan["zb"])
    nc = _CACHE[key]
    r = run_bass_kernel_spmd(nc, in_maps, core_ids=list(range(NCORES)))
    out = np.zeros((NS, C3), np.float32)
    s_assign = plan["s_assign"]
    for c in range(NCORES):
        oc = r.results[c]["out_own"]
        for slot in range(S_PER_CORE):
            t = s_assign[slot, c]
            out[t * P:(t + 1) * P] = oc[slot * P:(slot + 1) * P]
    return out
